# revision 25
# baseline (speedup 1.0000x reference)
"""Trainium2 Bass kernel for nn_MultiHeadAttention (b=2, n=4096, d=512, h=8, hd=64).

Sharding: 8 cores; core c handles batch b=c//4 and head pair j=c%4
(heads 2j, 2j+1). Tensor-parallel heads: each core computes a partial
output-projection y_part; host sums the 4 partials per batch and adds bo.

Per-core pipeline (all matmuls float32r = full PE speed, ~tf32 precision):
  x[b] -> PE-transpose -> xT
  Q/K projected for both heads at once, then replicated into per-head
    top/bottom-identical layouts (SBUF->SBUF DMA) so the K=64 scoresT
    matmuls row-pack two key-chunks concurrently on the PE array
  scoresT [keys,q] supers of 3 PSUM banks -> one Exp ACTIVATE per super
    (scale 1/8 folded into the activation, output f32r)
  attn@v with ones-augmented V (M=65): row 64 accumulates softmax sums
  per-q-block normalize (reciprocal + DRAM-roundtrip partition broadcast)
  y_part = OT.T @ WoT (K=128 over both heads), deferred one q-block

Scheduling: engines are in-order, so program order is shaped for overlap:
  - the first (h0,qb0) sweep's supers are interleaved into phase 1 as the
    k-blocks they need are projected, so ACT starts exp'ing early;
  - phase 2 is software-pipelined: scores of super s+1 are emitted BEFORE
    attn@v of super s, so PE computes the next scores while ACT exp's the
    current super instead of serializing exp -> attnv -> scores.
"""

import numpy as np

B, N, D, H, HD = 2, 4096, 512, 8, 64
NT = N // 128          # 32 n-tiles
NBLK = N // 512        # 8 n-blocks
KC = N // 128          # 32 key chunks
QB = N // 512          # 8 q-blocks
SUPERS = []
_c = 0
while _c < KC:
    w = min(3, KC - _c)
    SUPERS.append((_c, w))
    _c += w

_CACHE = {}
ABLATE = "base"  # timing-ablation knob, used only by scratch/ablate.py


def _build_nc(loop_n=None):
    """Build the SPMD kernel. loop_n wraps the body in a hardware For loop
    (used only for timing amplification, never for the graded path)."""
    import contextlib

    import concourse.bass as bass
    import concourse.mybir as mybir
    import concourse.tile as tile
    from concourse import bacc

    F32 = mybir.dt.float32
    F32R = mybir.dt.float32r
    EXP = mybir.ActivationFunctionType.Exp

    nc = bacc.Bacc("TRN2", target_bir_lowering=False, debug=False, num_devices=8)

    xb_d = nc.dram_tensor("xb", [N, D], F32R, kind="ExternalInput")
    w_d = {}
    for nm in ("wq2", "wk2", "wv"):
        w_d[nm] = nc.dram_tensor(nm, [128, 4, 128], F32R, kind="ExternalInput")
    woT_d = nc.dram_tensor("woT", [128, 512], F32R, kind="ExternalInput")
    ones_d = nc.dram_tensor("ones", [128, 1], F32R, kind="ExternalInput")
    ident_d = nc.dram_tensor("ident", [128, 128], F32R, kind="ExternalInput")
    y_d = nc.dram_tensor("y_part", [N, D], F32, kind="ExternalOutput")
    recip_dram = nc.dram_tensor("recip_scratch", [2, N], F32, kind="Internal")

    with tile.TileContext(nc) as tc:
        with (
            tc.tile_pool(name="singles", bufs=1) as singles,
            tc.tile_pool(name="sb_x", bufs=4) as sb_x,
            tc.tile_pool(name="sb_xT", bufs=3) as sb_xT,
            tc.tile_pool(name="sb_vt", bufs=2) as sb_vt,
            tc.tile_pool(name="sb_exp", bufs=3) as sb_exp,
            tc.tile_pool(name="sb_scr", bufs=2) as sb_scr,
            tc.tile_pool(name="sb_y", bufs=3) as sb_y,
            tc.tile_pool(name="ps3", bufs=2, space="PSUM") as ps3,
            tc.tile_pool(name="ps1", bufs=2, space="PSUM") as ps1,
        ):
            loop_ctx = (
                tc.For_i(0, loop_n, 1) if loop_n else contextlib.nullcontext()
            )
            with loop_ctx:
                ident = singles.tile([128, 128], F32R)
                nc.sync.dma_start(out=ident, in_=ident_d.ap())
                # warm the ACT Exp table while phase 1 runs
                warm = singles.tile([1, 1], F32)
                nc.scalar.activation(out=warm, in_=ident[0:1, 0:1], func=EXP)
                wt = {}
                for nm in ("wq2", "wk2", "wv"):
                    wt[nm] = singles.tile(
                        [128, 4, 128], F32R, tag=f"w_{nm}", name=f"wt_{nm}"
                    )
                    nc.sync.dma_start(out=wt[nm], in_=w_d[nm].ap())
                woT = singles.tile([128, 512], F32R)
                nc.sync.dma_start(out=woT, in_=woT_d.ap())

                qrep = [singles.tile([128, N], F32R, tag=f"qrep{h}", name=f"qrep{h}")
                        for h in range(2)]
                krep = [singles.tile([128, N], F32R, tag=f"krep{h}", name=f"krep{h}")
                        for h in range(2)]
                v_aug = [singles.tile([128, KC, 65], F32R, tag=f"vaug{h}",
                                      name=f"vaug{h}") for h in range(2)]
                ot2 = singles.tile([128, N], F32R)
                recip_b = singles.tile([128, N], F32)

                # ones column of v_aug: one small DMA, then DVE broadcasts
                # along the free dim (step-0 read AP)
                ones_sb = singles.tile([128, 1], F32R)
                nc.sync.dma_start(out=ones_sb, in_=ones_d.ap())
                for h in range(2):
                    ones_rd = bass.AP(
                        tensor=ones_sb.tensor, offset=ones_sb.offset,
                        ap=[ones_sb.ap[0], [0, KC], [1, 1]],
                    )
                    nc.vector.tensor_copy(out=v_aug[h][:, :, HD:65], in_=ones_rd)

                xa = xb_d.ap()

                # -------- attention helpers --------
                def finalize(qb):
                    qs = slice(qb * 512, (qb + 1) * 512)
                    nc.vector.tensor_mul(ot2[:, qs], ot2[:, qs], recip_b[:, qs])
                    for nt in range(4 * qb, 4 * qb + 4):
                        psy = ps1.tile([128, 512], F32, tag="psA", name="psy")
                        nc.tensor.matmul(
                            psy, ot2[:, nt * 128:(nt + 1) * 128], woT,
                            start=True, stop=True,
                        )
                        yb = sb_y.tile([128, 512], F32, tag="yb", name="yb")
                        nc.vector.tensor_copy(out=yb, in_=psy)
                        nc.sync.dma_start(
                            out=y_d.ap()[nt * 128:(nt + 1) * 128, :], in_=yb
                        )

                def scores_exp(h, qb, s0, w):
                    qs = slice(qb * 512, (qb + 1) * 512)
                    ps_s = ps3.tile([128, 3, 512], F32, tag="ps_s", name="ps_s")
                    for i in range(w):
                        c = s0 + i
                        if ABLATE == "no_packing":
                            half = slice(0, 64)
                        else:
                            half = slice((c % 2) * 64, (c % 2) * 64 + 64)
                        nc.tensor.matmul(
                            ps_s[:, i, :],
                            krep[h][half, c * 128:(c + 1) * 128],
                            qrep[h][half, qs],
                            start=True, stop=True,
                        )
                    expT = sb_exp.tile([128, 3, 512], F32R, tag="expT", name="expT")
                    sl = slice(0, 1) if ABLATE == "tiny_exp" else slice(0, 512)
                    nc.scalar.activation(
                        out=expT[:, 0:w, sl], in_=ps_s[:, 0:w, sl],
                        func=EXP, scale=0.125,
                    )
                    return expT

                def attnv(h, qb, ps_o, expT, s0, w):
                    for i in range(w):
                        c = s0 + i
                        if ABLATE == "no_attnv" and c > 0:
                            continue
                        nc.tensor.matmul(
                            ps_o[0:65, :], v_aug[h][:, c, :], expT[:, i, :],
                            start=(c == 0),
                            stop=(c == (0 if ABLATE == "no_attnv" else KC - 1)),
                        )

                def sweep_tail(h, qb, ps_o):
                    qs = slice(qb * 512, (qb + 1) * 512)
                    if h == 1 and qb > 0:
                        finalize(qb - 1)
                    scr = sb_scr.tile([65, 512], F32R, tag="scr", name="scr")
                    nc.vector.tensor_copy(out=scr, in_=ps_o[0:65, :])
                    nc.sync.dma_start(
                        out=ot2[h * 64:(h + 1) * 64, qs], in_=scr[0:64, :]
                    )
                    # reciprocal of softmax sums in place, bounce via DRAM
                    # to broadcast across partitions
                    rrow = scr[64:65, :].bitcast(F32)
                    nc.vector.reciprocal(out=rrow, in_=rrow)
                    nc.sync.dma_start(out=recip_dram.ap()[h:h + 1, qs], in_=rrow)
                    rb = bass.AP(
                        tensor=recip_dram, offset=h * N + qb * 512,
                        ap=[[0, 64], [1, 512]],
                    )
                    nc.sync.dma_start(out=recip_b[h * 64:(h + 1) * 64, qs], in_=rb)

                # supers of the first sweep become ready when the k-block
                # holding their last chunk has been projected
                ready = {}
                for s0, w in SUPERS:
                    ready.setdefault((s0 + w - 1) // 4, []).append((s0, w))
                ps_o00 = ps1.tile([128, 512], F32, tag="psA", name="ps_o00")

                # ---- phase 1: load x, transpose, project ----
                for jj in range(NBLK):
                    xT_blk = sb_xT.tile([128, 4, 512], F32R, tag="xT", name="xT_blk")
                    for tt in range(4):
                        t = 4 * jj + tt
                        x_t = sb_x.tile([128, 512], F32R, tag="x", name="x_t")
                        nc.sync.dma_start(out=x_t, in_=xa[t * 128:(t + 1) * 128, :])
                        pst = ps3.tile([128, 4, 128], F32R, tag="ps_s", name="pst")
                        for dc in range(4):
                            nc.tensor.transpose(
                                pst[:, dc, :], x_t[:, dc * 128:(dc + 1) * 128], ident
                            )
                        nc.vector.tensor_copy(
                            out=xT_blk[:, :, tt * 128:(tt + 1) * 128], in_=pst
                        )
                    ns = slice(jj * 512, (jj + 1) * 512)
                    # Q and K for both heads; drain split per head, then
                    # replicate the other half of each rep tensor via DMA
                    for pi, (nm, rep) in enumerate((("wq2", qrep), ("wk2", krep))):
                        pool_p = ps3 if pi % 2 else ps1
                        psp = pool_p.tile(
                            [128, 512], F32,
                            tag="ps_s" if pi % 2 else "psA", name="psp",
                        )
                        for dc in range(4):
                            nc.tensor.matmul(
                                psp, wt[nm][:, dc, :], xT_blk[:, dc, :],
                                start=(dc == 0), stop=(dc == 3),
                            )
                        nc.vector.tensor_copy(out=rep[0][0:64, ns], in_=psp[0:64, :])
                        nc.vector.tensor_copy(out=rep[1][64:128, ns],
                                              in_=psp[64:128, :])
                        nc.sync.dma_start(out=rep[0][64:128, ns], in_=rep[0][0:64, ns])
                        nc.sync.dma_start(out=rep[1][0:64, ns], in_=rep[1][64:128, ns])
                    # V for both heads, then transpose into v_aug
                    psp = ps1.tile([128, 512], F32, tag="psA", name="psp_v")
                    for dc in range(4):
                        nc.tensor.matmul(
                            psp, wt["wv"][:, dc, :], xT_blk[:, dc, :],
                            start=(dc == 0), stop=(dc == 3),
                        )
                    vt_blk = sb_vt.tile([128, 512], F32R, tag="vt", name="vt_blk")
                    nc.vector.tensor_copy(out=vt_blk, in_=psp)
                    for h in range(2):
                        psv = ps1.tile([128, 4, 64], F32R, tag="psA", name="psv")
                        for tt in range(4):
                            nc.tensor.transpose(
                                psv[:, tt, :],
                                vt_blk[h * 64:(h + 1) * 64, tt * 128:(tt + 1) * 128],
                                ident[h * 64:(h + 1) * 64, h * 64:(h + 1) * 64],
                            )
                        nc.vector.tensor_copy(
                            out=v_aug[h][:, 4 * jj:4 * jj + 4, 0:HD], in_=psv
                        )
                    # interleave the first sweep's ready supers into phase 1
                    if ABLATE != "phase1_only":
                        for s0, w in ready.get(jj, []):
                            expT = scores_exp(0, 0, s0, w)
                            attnv(0, 0, ps_o00, expT, s0, w)

                # ---- phase 2: remaining sweeps, software-pipelined on PE ----
                if ABLATE == "phase1_only":
                    units = []
                else:
                    sweep_tail(0, 0, ps_o00)
                if ABLATE != "phase1_only":
                    units = []
                    for h in range(2):
                        for qb in range(QB):
                            if (h, qb) == (0, 0):
                                continue
                            for si, (s0, w) in enumerate(SUPERS):
                                units.append((h, qb, s0, w, si == len(SUPERS) - 1))

                ps_o_cur = {}
                pending = [None]

                def flush_pending():
                    h, qb, s0, w, last, expT = pending[0]
                    if (h, qb) not in ps_o_cur:
                        ps_o_cur[(h, qb)] = ps1.tile(
                            [128, 512], F32, tag="psA", name="ps_o"
                        )
                    attnv(h, qb, ps_o_cur[(h, qb)], expT, s0, w)
                    if last:
                        sweep_tail(h, qb, ps_o_cur.pop((h, qb)))

                for h, qb, s0, w, last in units:
                    expT = scores_exp(h, qb, s0, w)
                    if pending[0] is not None:
                        flush_pending()
                    pending[0] = (h, qb, s0, w, last, expT)
                if ABLATE != "phase1_only":
                    flush_pending()
                    finalize(QB - 1)

    nc.compile()
    return nc


def _prep_in_maps(x, Wq, Wk, Wv, Wo):
    x = np.asarray(x, dtype=np.float32)
    Wq = np.asarray(Wq, dtype=np.float32)
    Wk = np.asarray(Wk, dtype=np.float32)
    Wv = np.asarray(Wv, dtype=np.float32)
    Wo = np.asarray(Wo, dtype=np.float32)
    ones = np.ones((128, 1), np.float32)
    ident = np.eye(128, dtype=np.float32)
    in_maps = []
    for c in range(8):
        b, j = c // 4, c % 4
        rows = slice(128 * j, 128 * (j + 1))
        m = {
            "xb": np.ascontiguousarray(x[b]),
            "ones": ones,
            "ident": ident,
            "woT": np.ascontiguousarray(Wo[:, rows].T),
        }
        for nm, W in (("wq2", Wq), ("wk2", Wk), ("wv", Wv)):
            A = W[rows]                     # [128, 512] rows = h1(64) | h2(64)
            # lhsT layout [p=d-within-chunk, c=d-chunk, k=out-col], contiguous
            m[nm] = np.ascontiguousarray(
                A.reshape(128, 4, 128).transpose(2, 1, 0)
            )
        in_maps.append(m)
    return in_maps


def kernel(x, Wq, Wk, Wv, Wo, bo):
    from concourse.bass_utils import run_bass_kernel_spmd

    if "nc" not in _CACHE:
        _CACHE["nc"] = _build_nc()
    nc = _CACHE["nc"]
    in_maps = _prep_in_maps(x, Wq, Wk, Wv, Wo)
    res = run_bass_kernel_spmd(nc, in_maps, core_ids=list(range(8)))
    y = np.zeros((B, N, D), np.float32)
    for c in range(8):
        y[c // 4] += res.results[c]["y_part"]
    y += np.asarray(bo, dtype=np.float32)[None, None, :]
    return y
